# revision 30
# baseline (speedup 1.0000x reference)
"""GCN message-passing kernel for Trainium2 (8 NeuronCores, SPMD).

out = (D^-1/2 (A+I) D^-1/2 X) W^T + b   for a random graph with
N=100000 nodes, E=1600000 edges, 128 channels.

Strategy (per core; destinations sharded 12500 nodes/core):
- Host pre-scales x by dinv (y = dinv * x, bf16). Every edge becomes a
  "token"; aggregation for a window of 128 destinations is
  aggT[ch, dst] = sum_tokens y[src]^T * onehot,
  onehot[e, d] = (destrel[e] == d), computed as bf16 matmuls
  accumulated in fp32 PSUM (lhsT = gathered message tile, rhs = one-hot).
- Tokens are packed CONTIGUOUSLY per (super, src-bank) region (window
  capacity = max count over cores, no per-window tile rounding); a tile
  straddling a window boundary is consumed by one matmul per window it
  touches ("uses"), each with its own one-hot column built from a per-use
  destrel table (fp16; non-members hold 512 which never matches iota 0..127).
  This cuts SWDGE gather descriptors ~9% - the kernel is descriptor-
  generation-bound (~300 descs/us/core through the GpSimd SWDGE queues).
- One dma_gather call per region (~68 calls) round-robins the 4 SWDGE
  queues; the full idx table, destrel table and self rows are preloaded
  into SBUF so gathers never wait on metadata.
- Self-loops skip the gather: window w's own y rows come from the resident
  xself tile and enter the same one-hot matmul path (constant one-hot).
- Finalize per super: Z^T = W^T @ aggT for 3 windows per matmul
  (rhs [128, 384]); host transposes, applies dinv[dst] and bias.
"""

import sys

sys.path.insert(0, "/opt/trn_rl_repo")
import numpy as np

N = 100000
D = 128
CORES = 8
NPC = N // CORES  # 12500 dests per core
NW = (NPC + 127) // 128  # 98 windows per core
SUP = 6  # windows per super (PSUM accumulator banks: 6 + 2 for out matmul)
NSUP = (NW + SUP - 1) // SUP  # 17 supers
BANK = 32768
NBANKS = (N + BANK - 1) // BANK  # 4
OHK = 16  # one-hot batch, in uses
PAD_DR = 512.0  # fp16-exact, never matches iota 0..127


def _schedule(win_cap):
    """Build the contiguous packing schedule from per-window capacities.

    win_cap: [NSUP, NBANKS, NW] int - max-over-cores token count per group.
    Tokens are packed per (super, bank) region; the global tile stream is
    ordered PAIR-major ([super-pair][bank][super][window]) so one dma_gather
    call covers two supers' regions of the same bank (~6400 descriptors -
    larger calls amortize the SWDGE per-call dead time).
    Use numbering follows CONSUMPTION order (super-major, bank-inner).
    """
    sup_windows = [list(range(s * SUP, min((s + 1) * SUP, NW))) for s in range(NSUP)]
    regions = {}
    mm_total = np.ones(NW, dtype=np.int64)
    u_global = 0
    for s in range(NSUP):
        wins = sup_windows[s]
        for b in range(NBANKS):
            off = 0
            uses = []
            offs = {}
            for w in wins:
                cap = int(win_cap[s, b, w])
                offs[w] = off
                if cap > 0:
                    t_lo = off // 128
                    t_hi = (off + cap - 1) // 128
                    for t in range(t_lo, t_hi + 1):
                        uses.append((w, t, u_global, off))
                        u_global += 1
                        mm_total[w] += 1
                off += cap
            regions[(s, b)] = {
                "ntiles": (off + 127) // 128,
                "uses": uses,
                "offs": offs,
            }
    tile_base = 0
    for s in range(NSUP):
        for b in range(NBANKS):
            regions[(s, b)]["tile_base"] = tile_base
            tile_base += regions[(s, b)]["ntiles"]
    return sup_windows, regions, mm_total, tile_base, u_global


def _build_bass(win_cap):
    """Build the SPMD Bass program from the packing schedule."""
    import concourse.mybir as mybir
    import concourse.tile as tile
    from concourse import bacc

    sup_windows, regions, mm_total, T_total, U_total = _schedule(win_cap)
    NTOK = 128 * T_total
    RMAX = max(r["ntiles"] for r in regions.values())
    BF16 = mybir.dt.bfloat16
    FP16 = mybir.dt.float16

    nc = bacc.Bacc(None, target_bir_lowering=False, num_swdge_queues=4)
    xt = nc.dram_tensor("xt", [N, D], BF16, kind="ExternalInput")
    idxs = nc.dram_tensor("idxs", [128, NTOK // 16], mybir.dt.int16, kind="ExternalInput")
    destrel = nc.dram_tensor("destrel", [128, U_total], FP16, kind="ExternalInput")
    wt = nc.dram_tensor("wt", [D, D], BF16, kind="ExternalInput")
    outT = nc.dram_tensor("outT", [D, NW * 128], mybir.dt.float32, kind="ExternalOutput")
    xself_in = nc.dram_tensor("xself", [128, NW * D], BF16, kind="ExternalInput")

    xviews = [xt[b * BANK : min((b + 1) * BANK, N), :] for b in range(NBANKS)]

    gq = [0]
    with tile.TileContext(nc) as tc:
        with (
            tc.tile_pool(name="const", bufs=1) as cpool,
            tc.tile_pool(name="meta", bufs=1) as mpool,
            tc.tile_pool(name="gp", bufs=10) as gpool,
            tc.tile_pool(name="ohp", bufs=8) as ohpool,
            tc.tile_pool(name="rhp", bufs=3) as rhpool,
            tc.tile_pool(name="outp", bufs=2) as outpool,
            tc.tile_pool(name="ps", bufs=1, space="PSUM") as pspool,
            tc.tile_pool(name="pso", bufs=2, space="PSUM") as psopool,
        ):
            # idx table first: gathers depend only on it. Split the preload so
            # the first regions' gathers unblock before the whole table lands.
            NSPLIT = 4
            # split at region boundaries (each gather call reads one region)
            bounds = sorted(r["tile_base"] for r in regions.values()) + [T_total]
            split_tiles = []
            prev = 0
            for j in range(1, NSPLIT):
                target = T_total * j // NSPLIT
                cut = min(bounds, key=lambda x: abs(x - target))
                if cut > prev:
                    split_tiles.append((prev, cut))
                    prev = cut
            split_tiles.append((prev, T_total))
            idx_parts = []
            for lo, hi in split_tiles:
                part = mpool.tile([128, (hi - lo) * 8], mybir.dt.int16)
                nc.sync.dma_start(out=part[:], in_=idxs[:, lo * 8 : hi * 8])
                idx_parts.append((lo, hi, part))

            def idx_slice(gs, ntiles):
                for lo, hi, part in idx_parts:
                    if lo <= gs and gs + ntiles <= hi:
                        return part[:, (gs - lo) * 8 : (gs - lo + ntiles) * 8]
                raise AssertionError("gather call crosses idx split")
            destrel_t = mpool.tile([128, U_total], FP16)
            nc.sync.dma_start(out=destrel_t[:], in_=destrel[:])
            xself_t = mpool.tile([128, NW * D], BF16)
            nc.sync.dma_start(out=xself_t[:], in_=xself_in[:])
            wt_t = cpool.tile([D, D], BF16)
            nc.sync.dma_start(out=wt_t[:], in_=wt[:])

            iota_f = cpool.tile([128, 128], mybir.dt.float32)
            nc.gpsimd.iota(
                iota_f[:], pattern=[[1, 128]], base=0, channel_multiplier=0,
                allow_small_or_imprecise_dtypes=True,
            )
            iota_t = cpool.tile([128, 128], FP16)
            nc.vector.tensor_copy(out=iota_t[:], in_=iota_f[:])
            pidx_f = cpool.tile([128, 1], mybir.dt.float32)
            nc.gpsimd.iota(
                pidx_f[:], pattern=[[1, 1]], base=0, channel_multiplier=1,
                allow_small_or_imprecise_dtypes=True,
            )
            pidx_t = cpool.tile([128, 1], FP16)
            nc.vector.tensor_copy(out=pidx_t[:], in_=pidx_f[:])
            selfbase_t = cpool.tile([128, 128], BF16)
            nc.vector.tensor_tensor(
                out=selfbase_t[:],
                in0=iota_t[:],
                in1=pidx_t[:, 0:1].to_broadcast([128, 128]),
                op=mybir.AluOpType.is_equal,
            )

            oh_batches = {}

            def oh_for(u):
                bnum = u // OHK
                if bnum not in oh_batches:
                    u0 = bnum * OHK
                    k = min(OHK, U_total - u0)
                    ohb = ohpool.tile([128, OHK, 128], BF16, tag="ohb")
                    nc.vector.tensor_tensor(
                        out=ohb[:, :k, :],
                        in0=iota_t[:, None, :].to_broadcast([128, k, 128]),
                        in1=destrel_t[:, u0 : u0 + k, None].to_broadcast([128, k, 128]),
                        op=mybir.AluOpType.is_equal,
                    )
                    oh_batches[bnum] = ohb
                    for old in list(oh_batches):
                        if old < bnum - 3:
                            del oh_batches[old]
                return oh_batches[bnum][:, u % OHK, :]

            for S in range(NSUP):
                if True:
                    wins = sup_windows[S]
                    psbank = {}
                    mm_done = {}
                    for w in wins:
                        ps = pspool.tile(
                            [128, 128], mybir.dt.float32, tag=f"psw{w % SUP}",
                            name=f"psw{S}_{w % SUP}",
                        )
                        psbank[w] = ps
                        # aggT[ch, dst]: short finalize (no transpose needed)
                        nc.tensor.matmul(
                            out=ps[:],
                            lhsT=xself_t[:, w * D : (w + 1) * D],
                            rhs=selfbase_t[:],
                            start=True,
                            stop=(mm_total[w] == 1),
                            skip_group_check=True,
                        )
                        mm_done[w] = 1
                    for b in range(NBANKS):
                        reg = regions[(S, b)]
                        if reg["ntiles"] == 0:
                            continue
                        gtile = gpool.tile([128, RMAX, D], BF16, tag="g")
                        gs = reg["tile_base"]
                        nc.gpsimd.dma_gather(
                            gtile[:, : reg["ntiles"], :],
                            xviews[b],
                            idx_slice(gs, reg["ntiles"]),
                            128 * reg["ntiles"],
                            128 * reg["ntiles"],
                            D,
                            elem_step=D,
                            single_packet=False,
                            queue_num=gq[0] % 4,
                        )
                        gq[0] += 1
                        toff = 0
                        for w, t, u, off in reg["uses"]:
                            nc.tensor.matmul(
                                out=psbank[w][:],
                                lhsT=gtile[:, toff + t, :],
                                rhs=oh_for(u),
                                start=False,
                                stop=(mm_done[w] == mm_total[w] - 1),
                                skip_group_check=True,
                            )
                            mm_done[w] += 1

                    # finalize super: Z^T = W^T @ aggT, 3 windows per matmul
                    nwin = len(wins)
                    ostage = outpool.tile(
                        [128, SUP * 128], mybir.dt.float32, tag="ostage"
                    )
                    for g0 in range(0, nwin, 3):
                        gw = wins[g0 : g0 + 3]
                        rh = rhpool.tile([128, 3 * 128], BF16, tag="rh")
                        for j, w in enumerate(gw):
                            nc.scalar.activation(
                                out=rh[:, j * 128 : (j + 1) * 128],
                                in_=psbank[w][:],
                                func=mybir.ActivationFunctionType.Identity,
                                scale=1.0,
                            )
                        mm = psopool.tile([D, 3 * 128], mybir.dt.float32, tag="po")
                        k = len(gw) * 128
                        nc.tensor.matmul(
                            out=mm[:, :k], lhsT=wt_t[:], rhs=rh[:, :k],
                            start=True, stop=True,
                        )
                        nc.scalar.activation(
                            out=ostage[:, g0 * 128 : g0 * 128 + k],
                            in_=mm[:, :k],
                            func=mybir.ActivationFunctionType.Identity,
                            scale=1.0,
                        )
                    nc.sync.dma_start(
                        out=outT[:, wins[0] * 128 : (wins[-1] + 1) * 128],
                        in_=ostage[:, : nwin * 128],
                    )

    nc.finalize()
    return nc


def _preprocess(x, edge_index, W, b):
    """Host-side sharding: build per-core token tables + global schedule."""
    import ml_dtypes

    row = np.asarray(edge_index[0], dtype=np.int64)
    col = np.asarray(edge_index[1], dtype=np.int64)
    deg = (np.bincount(col, minlength=N) + 1).astype(np.float32)
    dinv = deg**-0.5  # float32, deg >= 1 always

    core = row // NPC
    lrow = row - core * NPC
    w = lrow // 128
    drel = (lrow % 128).astype(np.float32)
    S = w // SUP
    beta = col // BANK
    crel = (col - beta * BANK).astype(np.int16)

    order = np.lexsort((w, beta, S, core))
    core_s = core[order]
    S_s = S[order]
    beta_s = beta[order]
    w_s = w[order]
    drel_s = drel[order]
    crel_s = crel[order]

    gid = ((core_s * NSUP + S_s) * NBANKS + beta_s) * NW + w_s
    NG = CORES * NSUP * NBANKS * NW
    counts = np.bincount(gid, minlength=NG).reshape(CORES, NSUP, NBANKS, NW)
    win_cap = counts.max(axis=0)  # [NSUP, NBANKS, NW]

    sup_windows, regions, mm_total, T_total, U_total = _schedule(win_cap)
    NTOK = 128 * T_total

    # token placement: global token index for (s,b,w,rank)
    tok_base = np.zeros((NSUP, NBANKS, NW), dtype=np.int64)
    for (s, b_), reg in regions.items():
        for ww, off in reg["offs"].items():
            tok_base[s, b_, ww] = reg["tile_base"] * 128 + off
    # use index lookup: (s,b,w,t_local) -> u
    use_of = {}
    for (s, b_), reg in regions.items():
        for ww, t, u, off in reg["uses"]:
            use_of[(s, b_, ww, t)] = u

    yf = (np.asarray(x, dtype=np.float32) * dinv[:, None]).astype(ml_dtypes.bfloat16)
    Wt = np.ascontiguousarray(np.asarray(W, dtype=np.float32).T).astype(
        ml_dtypes.bfloat16
    )

    gid_full = core_s * (NSUP * NBANKS * NW) + (S_s * NBANKS + beta_s) * NW + w_s
    uniq, first_idx, cnt = np.unique(gid_full, return_index=True, return_counts=True)
    rank = np.arange(len(gid_full)) - np.repeat(first_idx, cnt)
    pos = tok_base[S_s, beta_s, w_s] + rank  # global token position

    # per-token use index: tile t_local = (off+rank)//128 - tile_base... compute
    reg_tile_base = np.zeros((NSUP, NBANKS), dtype=np.int64)
    for (s, b_), reg in regions.items():
        reg_tile_base[s, b_] = reg["tile_base"]
    t_local = pos // 128 - reg_tile_base[S_s, beta_s]
    u_arr = np.empty(len(pos), dtype=np.int64)
    # vectorized-ish lookup via dict (1.6M entries, loop in C via map)
    keys = list(zip(S_s.tolist(), beta_s.tolist(), w_s.tolist(), t_local.tolist()))
    u_arr[:] = [use_of[k] for k in keys]

    core_bounds = np.searchsorted(core_s, np.arange(CORES + 1))
    in_maps = []
    for k in range(CORES):
        lo, hi = core_bounds[k], core_bounds[k + 1]
        idx16 = np.zeros(NTOK, dtype=np.int16)
        dr = np.full((128, U_total), PAD_DR, dtype=np.float32)
        p = pos[lo:hi]
        idx16[p] = crel_s[lo:hi]
        dr[p % 128, u_arr[lo:hi]] = drel_s[lo:hi]
        idx_tile = np.tile(idx16.reshape(-1, 16).T, (8, 1))  # [128, NTOK//16]
        dr_t = dr.astype(np.float16)

        xs = np.zeros((NW * 128, D), dtype=ml_dtypes.bfloat16)
        xs[:NPC] = yf[k * NPC : (k + 1) * NPC]
        xs_t = np.ascontiguousarray(
            xs.reshape(NW, 128, D).transpose(1, 0, 2)
        ).reshape(128, NW * D)
        in_maps.append(
            {
                "xt": yf,
                "idxs": idx_tile,
                "destrel": dr_t,
                "xself": xs_t,
                "wt": Wt,
            }
        )

    return win_cap, in_maps, dinv


_CACHE = {}


def kernel(x, edge_index, W, b, _want_trace=False):
    from concourse.bass_utils import run_bass_kernel_spmd

    win_cap, in_maps, dinv = _preprocess(x, edge_index, W, b)
    key = win_cap.tobytes()
    if key not in _CACHE:
        _CACHE[key] = _build_bass(win_cap)
    nc = _CACHE[key]

    kwargs = {}
    if _want_trace:
        kwargs = dict(trace=True, trace_cores=list(range(CORES)))
    res = run_bass_kernel_spmd(nc, in_maps, core_ids=list(range(CORES)), **kwargs)

    bv = np.asarray(b, dtype=np.float32)[None, :]
    out = np.empty((N, D), dtype=np.float32)
    for k in range(CORES):
        z = res.results[k]["outT"][:, :NPC].T  # [NPC, D] = agg @ W^T
        out[k * NPC : (k + 1) * NPC] = (
            dinv[k * NPC : (k + 1) * NPC, None] * z + bv
        )
    if _want_trace:
        return out, res
    return out
